# revision 21
# baseline (speedup 1.0000x reference)
"""Grouped MLP (MoE, 8 experts, SwiGLU) — expert-parallel Bass kernel for 8 TRN2 cores.

Reference computation (per expert e, T=1024 tokens each):
    fc1  = x_e @ w1_e            # [T, 2F]
    gate, val = split(fc1)       # [T, F] each
    act  = silu(gate) * val      # [T, F]
    out  = act @ w2_e            # [T, H]

Sharding: expert-parallel — core e owns expert e entirely. No collectives.

Two-phase per-core layout (v3):
  * Phase 1 (mm1): computes fc1^T per f-block (stationary = w1 block [h,f],
    moving = x^T), swiglu on ACT+DVE, act^T tiles stay resident in SBUF.
  * Phase 2 (mm2): H split into 4 512-col quarters; per quarter, stream w2
    tiles and accumulate all 8 token-blocks in 8 PSUM banks over all 64
    f-blocks, then DMA each [128,512] PSUM tile straight to DRAM.
  * v3: N2 adjacent f-block pairs of mm2 run as fp8e4 DoubleRow matmuls
    (2 contraction blocks per PE instruction = 2x throughput on those
    blocks).  Operands are scale-compensated (w2*S stationary, act/S
    moving, S=8) so each plane's product is exact in scale and both
    operands sit in e4m3's sweet range; the raw e4m3 noise on those
    blocks costs ~1.7e-2 of the 2e-2 error budget (error is fp16-exact
    elsewhere).  First two w2 tiles of phase 2 prefetch during the last
    phase-1 iteration to close the phase-boundary PE gap.
"""

import numpy as np
from contextlib import ExitStack

import ml_dtypes

import concourse.bacc as bacc
import concourse.mybir as mybir
import concourse.tile as tile
from concourse.bass_utils import run_bass_kernel_spmd

E = 8          # experts == cores
T = 1024       # tokens per expert
H = 2048       # hidden
F = 8192       # ffn intermediate (act width)
NHB = H // 128   # 16 h-blocks (contraction blocks for mm1)
NFB = F // 128   # 64 f-blocks (act columns)
NTB = T // 128   # 8 token blocks
HB2 = NHB // 2   # 8 h-blocks per half
NQ = H // 512    # 4 output column quarters

F16 = mybir.dt.float16
F32 = mybir.dt.float32
F8 = mybir.dt.float8e4
NP8 = ml_dtypes.float8_e4m3

# mm2 fp8 DoubleRow config: N2 adjacent f-block pairs starting at f-block 4.
N2 = 5
FP8_PAIRS = [(4 + 2 * p, 5 + 2 * p) for p in range(N2)]
FP8_J = {j for pr in FP8_PAIRS for j in pr}
PAIR_OF = {pr[0]: i for i, pr in enumerate(FP8_PAIRS)}
S8 = 8.0       # w2 pre-scaled by S8 on host, act scaled by 1/S8 on chip

_CACHE: dict = {}


def build_nc():
    nc = bacc.Bacc(None, target_bir_lowering=False, debug=False, num_devices=E)

    xt_d = nc.declare_dram_parameter("xt", [128, NHB, T], F16, isOutput=False)
    w1_d = nc.declare_dram_parameter("w1t", [2 * NFB, 2, 128, HB2, 128], F16,
                                     isOutput=False)
    w2_d = nc.declare_dram_parameter("w2r", [NFB, 128, H], F16, isOutput=False)
    w28_d = nc.declare_dram_parameter("w28", [max(N2, 1), 128, 2, H], F8,
                                      isOutput=False)
    out_d = nc.declare_dram_parameter("out", [NHB, 128, T], F16, isOutput=True)

    with ExitStack() as ctx:
        tc = ctx.enter_context(tile.TileContext(nc))
        persist = ctx.enter_context(tc.tile_pool(name="persist", bufs=1))
        w1_pool = ctx.enter_context(tc.tile_pool(name="w1", bufs=2))
        silu_pool = ctx.enter_context(tc.tile_pool(name="silu", bufs=2))
        # Single-tag ring pools (one tag, bufs = live tile count): pool
        # init/teardown sync ops scale with tag count, so rings keep the
        # fixed head/tail short.
        xt_pool = ctx.enter_context(tc.tile_pool(name="xt", bufs=NHB))
        act_pool = ctx.enter_context(
            tc.tile_pool(name="act", bufs=NFB - 2 * N2))
        act8_pool = ctx.enter_context(
            tc.tile_pool(name="act8", bufs=max(N2, 1)))
        w2_pool = ctx.enter_context(tc.tile_pool(name="w2", bufs=4))

        prefetched = {}

        def fetch_j(j, qv=None):
            # w1 gate/val blocks, each split into h-halves so the first
            # LDWEIGHTS only waits on a 256KB transfer.  Gate on sync, val
            # on scalar spreads the steady-state w1 stream over both queues.
            if j in prefetched:
                return prefetched.pop(j)
            qv = qv or nc.scalar
            w1g_lo = w1_pool.tile([128, HB2, 128], F16, tag="w1g_lo")
            nc.sync.dma_start(w1g_lo[:], w1_d[j, 0])
            w1g_hi = w1_pool.tile([128, HB2, 128], F16, tag="w1g_hi")
            nc.sync.dma_start(w1g_hi[:], w1_d[j, 1])
            w1v_lo = w1_pool.tile([128, HB2, 128], F16, tag="w1v_lo")
            qv.dma_start(w1v_lo[:], w1_d[NFB + j, 0])
            w1v_hi = w1_pool.tile([128, HB2, 128], F16, tag="w1v_hi")
            qv.dma_start(w1v_hi[:], w1_d[NFB + j, 1])
            return ((w1g_lo, w1g_hi), (w1v_lo, w1v_hi))

        # Only sync+scalar issue DMAs: those queues are hardware-dynamic;
        # the gpsimd queue is software-dynamic (Q7-generated descriptors,
        # ~2-5x slower).  Prime both queues with tiny transfers so their
        # one-time warmup cost is paid before the real loads.
        prime = persist.tile([128, 2, 16], F16, tag="prime")
        nc.sync.dma_start(prime[:, 0, :], xt_d[:, 0, 0:16])
        nc.scalar.dma_start(prime[:, 1, :], xt_d[:, 1, 0:16])
        prime_g = persist.tile([128, 16], F16, tag="prime_g")
        nc.gpsimd.dma_start(prime_g[:], xt_d[:, 2, 0:16])

        # Zeroed tile for PE pre-warm matmuls (see phase 1).
        warm = persist.tile([128, 192], F16, tag="warm")
        nc.vector.memset(warm[:], 0.0)

        # First gate weights ahead of everything on sync; xt split across
        # both queues (evens on scalar so xt0 doesn't queue behind w1g).
        w1g_lo0 = w1_pool.tile([128, HB2, 128], F16, tag="w1g_lo")
        nc.sync.dma_start(w1g_lo0[:], w1_d[0, 0])
        w1g_hi0 = w1_pool.tile([128, HB2, 128], F16, tag="w1g_hi")
        nc.sync.dma_start(w1g_hi0[:], w1_d[0, 1])

        # xt[13]/xt[15] ride the (slow, software-dynamic) gpsimd queue as a
        # third stream: they unload 512KB from sync and still land before
        # the j0 h-loop reaches them.
        xt = []
        for h in range(NHB):
            xh = xt_pool.tile([128, T], F16, tag="x")
            if h in (13, 15):
                qx = nc.gpsimd
            else:
                qx = nc.scalar if h % 2 == 0 else nc.sync
            qx.dma_start(xh[:], xt_d[:, h, :])
            xt.append(xh)

        w1v_lo0 = w1_pool.tile([128, HB2, 128], F16, tag="w1v_lo")
        nc.scalar.dma_start(w1v_lo0[:], w1_d[NFB, 0])
        w1v_hi0 = w1_pool.tile([128, HB2, 128], F16, tag="w1v_hi")
        nc.scalar.dma_start(w1v_hi0[:], w1_d[NFB, 1])
        prefetched[0] = ((w1g_lo0, w1g_hi0), (w1v_lo0, w1v_hi0))
        prefetched[1] = fetch_j(1)

        act_tiles = {}     # j -> fp16 [128, T] tile (non-fp8 f-blocks)
        act8_tiles = {}    # pair index -> fp8 [128, 2, T] tile
        w2_prefetch = []   # phase-2 head tiles fetched during phase 1

        # One PSUM pool for both phases (2 bufs x (gate+val) = all 8 banks).
        # Phase 2 allocates through the same gate/val rings, so each tile
        # waits only on its own slot's last phase-1 reader instead of a
        # pool-exit barrier on the whole final swiglu chain.
        ps1 = ctx.enter_context(tc.tile_pool(name="ps", bufs=2, space="PSUM"))
        if True:
            # ~70 tiny matmuls on zeroes keep the PE busy while the first
            # real DMAs land, so the HAM clock-gate reaches 8/8 (2.4GHz)
            # before the first real matmul instead of ~3.4us into it.
            warm_ps = ps1.tile([128, T], F32, tag="gate", name="warm_ps")

            def warm_mms(n):
                for _ in range(n):
                    nc.tensor.matmul(warm_ps[0:64, 0:64], warm[:, 0:64],
                                     warm[:, 64:128], start=True, stop=True)

            warm_mms(28)
            for j in range(NFB):
                w1g, w1v = fetch_j(j)
                if j + 1 < NFB:
                    prefetched[j + 1] = fetch_j(j + 1)
                elif j == NFB - 1:
                    # Phase-2 prefetch: first two w2 tiles of quarter 0 so
                    # phase 2's first matmuls don't wait on the w2 stream.
                    for pj in (0, 1):
                        w2t = w2_pool.tile([128, 512], F16, tag="w2")
                        nc.sync.dma_start(w2t[:], w2_d[pj, :, 0:512])
                        w2_prefetch.append(w2t)

                gate_ps = ps1.tile([128, T], F32, tag="gate")
                for h in range(NHB):
                    st, sp = (h == 0), (h == NHB - 1)
                    wt = w1g[h // HB2][:, h % HB2, :]
                    nc.tensor.matmul(gate_ps[:, 0:512], wt, xt[h][:, 0:512],
                                     start=st, stop=sp)
                    nc.tensor.matmul(gate_ps[:, 512:1024], wt,
                                     xt[h][:, 512:1024], start=st, stop=sp)
                    if j == 0 and h < 2:
                        # Filler keeps the HAM clock-gate warm across the
                        # early xt DMA ramp stalls.
                        warm_mms(16)
                val_ps = ps1.tile([128, T], F32, tag="val")
                for h in range(NHB):
                    st, sp = (h == 0), (h == NHB - 1)
                    wt = w1v[h // HB2][:, h % HB2, :]
                    nc.tensor.matmul(val_ps[:, 0:512], wt, xt[h][:, 0:512],
                                     start=st, stop=sp)
                    nc.tensor.matmul(val_ps[:, 512:1024], wt,
                                     xt[h][:, 512:1024], start=st, stop=sp)

                # Native ACT-engine SiLU: act = silu(gate) * val in one
                # ACT op + one DVE mul.  The shorter DVE chain also frees
                # the gate/val PSUM slots sooner at the phase boundary.
                sl_sb = silu_pool.tile([128, T], F16, tag="sil")
                nc.scalar.activation(sl_sb[:], gate_ps[:],
                                     mybir.ActivationFunctionType.Silu)
                if j in FP8_J:
                    # fp8 f-block: store act/S8 as one plane of the pair's
                    # fp8 DoubleRow moving tile.  val/S8 via the ACT-engine
                    # input scale; DVE mul writes e4m3 directly.
                    vs_sb = silu_pool.tile([128, T], F16, tag="vs")
                    nc.scalar.activation(vs_sb[:], val_ps[:],
                                         mybir.ActivationFunctionType.Copy,
                                         scale=1.0 / S8)
                    if j in PAIR_OF:
                        a8 = act8_pool.tile([128, 2, T], F8, tag="a8")
                        act8_tiles[PAIR_OF[j]] = a8
                    else:
                        a8 = act8_tiles[PAIR_OF[j - 1]]
                    nc.vector.tensor_mul(a8[:, j % 2, :], sl_sb[:], vs_sb[:])
                else:
                    actt = act_pool.tile([128, T], F16, tag="a")
                    nc.vector.tensor_mul(actt[:], sl_sb[:], val_ps[:])
                    act_tiles[j] = actt

        # ---- Phase 2: mm2, transposed orientation ----
        # Stationary = w2 blocks [128f, 128h]; moving = the resident act^T
        # tiles.  Each stationary feeds 2 N=512 matmuls (token halves), so
        # LDWEIGHTS stays hidden AND w2 is read exactly once.  Output is
        # out^T per h-block; the host transposes for free.  Chunks of 4
        # h-blocks: 8 PSUM tiles (4hb x 2 token-halves) accumulate over all
        # 64 f-blocks, then PSUM->SBUF fp16 staging (ACT/DVE half each) and
        # out-DMA on the otherwise-idle scalar queue.  fp8 f-block pairs
        # run as single DoubleRow matmuls (2 contraction planes per instr).
        with tc.tile_pool(name="stage", bufs=4) as stage_pool:
            # Per chunk: 4 psum tiles [128,1024] via the phase-1 gate/val
            # rings; the natural ring order (j62 gate, j62 val, j63 gate,
            # j63 val) matches the order the swiglu DVE chain frees them,
            # and hbi order touches them earliest-freed first.
            NCH = NHB // 4
            for ch in range(NCH):
                raw = []
                for i in range(2):
                    raw.append(ps1.tile([128, T], F32, tag="gate",
                                        name=f"outg{ch}_{i}"))
                    raw.append(ps1.tile([128, T], F32, tag="val",
                                        name=f"outv{ch}_{i}"))
                outs = [raw[k // 2][:, (k % 2) * 512:(k % 2) * 512 + 512]
                        for k in range(8)]
                # In the last chunk, the final 4 f-blocks run h-block-major
                # so each h-block's PSUM tiles close out (and stage+DMA)
                # ~1.7us apart instead of all at the very end.
                last = (ch == NCH - 1)
                # Stagger: the last 2 (4 in the final chunk) f-blocks run
                # h-block-major so each h-block's PSUM closes and stages
                # early; the next chunk's first matmuls then don't wait on
                # this chunk's staging.  Stagger tiles ride the scalar queue
                # (only the out-DMA trickle there) to beat the sync backlog.
                jsplit = NFB - 4 if last else NFB - 2
                w2last = []
                for jj in range(jsplit, NFB):
                    w2t = w2_pool.tile([128, 512], F16, tag="w2l")
                    nc.scalar.dma_start(w2t[:],
                                        w2_d[jj, :, ch * 512:(ch + 1) * 512])
                    w2last.append(w2t)
                j = 0
                while j < jsplit:
                    st = (j == 0)
                    if j in PAIR_OF:
                        pi = PAIR_OF[j]
                        w28t = w2_pool.tile([128, 2, 512], F8, tag="w28")
                        nc.sync.dma_start(
                            w28t[:], w28_d[pi, :, :, ch * 512:(ch + 1) * 512])
                        a8 = act8_tiles[pi]
                        for hbi in range(4):
                            lhsT = w28t[:, :, hbi * 128:(hbi + 1) * 128]
                            nc.tensor.matmul(
                                outs[2 * hbi][:], lhsT, a8[:, :, 0:512],
                                start=st, stop=False,
                                perf_mode=mybir.MatmulPerfMode.DoubleRow)
                            nc.tensor.matmul(
                                outs[2 * hbi + 1][:], lhsT, a8[:, :, 512:1024],
                                start=st, stop=False,
                                perf_mode=mybir.MatmulPerfMode.DoubleRow)
                        j += 2
                        continue
                    if ch == 0 and j < len(w2_prefetch):
                        w2t = w2_prefetch[j]
                    else:
                        w2t = w2_pool.tile([128, 512], F16, tag="w2")
                        # ~79GB/s of w2 stream is borderline for one queue;
                        # put 3 of 8 tiles on scalar (which only carries the
                        # out-DMA trickle in phase 2).
                        q_w2 = nc.scalar if (j % 8) >= 5 else nc.sync
                        q_w2.dma_start(w2t[:],
                                       w2_d[j, :, ch * 512:(ch + 1) * 512])
                    sp = (j == NFB - 1)
                    for hbi in range(4):
                        lhsT = w2t[:, hbi * 128:(hbi + 1) * 128]
                        nc.tensor.matmul(outs[2 * hbi][:], lhsT,
                                         act_tiles[j][:, 0:512],
                                         start=st, stop=sp)
                        nc.tensor.matmul(outs[2 * hbi + 1][:], lhsT,
                                         act_tiles[j][:, 512:1024],
                                         start=st, stop=sp)
                    j += 1
                for hbi in range(4):
                    hb = ch * 4 + hbi
                    for jj in range(NFB - jsplit):
                        sp = (jj == NFB - jsplit - 1)
                        lhsT = w2last[jj][:, hbi * 128:(hbi + 1) * 128]
                        nc.tensor.matmul(outs[2 * hbi][:], lhsT,
                                         act_tiles[jsplit + jj][:, 0:512],
                                         start=False, stop=sp)
                        nc.tensor.matmul(outs[2 * hbi + 1][:], lhsT,
                                         act_tiles[jsplit + jj][:, 512:1024],
                                         start=False, stop=sp)
                    stg = stage_pool.tile([128, 1024], F16, tag="stage")
                    nc.scalar.activation(stg[:, 0:512], outs[2 * hbi][:],
                                         mybir.ActivationFunctionType.Copy)
                    nc.vector.tensor_copy(stg[:, 512:1024],
                                          outs[2 * hbi + 1][:])
                    q_out = nc.sync if last else nc.scalar
                    q_out.dma_start(out_d[hb], stg[:])

    nc.compile()
    return nc


def _get_nc():
    if "nc" not in _CACHE:
        _CACHE["nc"] = build_nc()
    return _CACHE["nc"]


def prep_inputs(permuted_hidden_states, w1, w2):
    """Host-side reshape/cast into the per-core DMA-friendly layouts."""
    x = np.asarray(permuted_hidden_states, dtype=np.float32)
    w1 = np.asarray(w1, dtype=np.float32)
    w2 = np.asarray(w2, dtype=np.float32)

    # xt[e][p, hb, t] = x[e*T + t, hb*128 + p]
    xt = np.ascontiguousarray(
        x.reshape(E, T, NHB, 128).transpose(0, 3, 2, 1).astype(np.float16))
    # w1t[e][jg, half, p, hb2, fi] = w1[e, (half*HB2+hb2)*128 + p, jg*128 + fi]
    w1t = np.ascontiguousarray(
        w1.reshape(E, 2, HB2, 128, 2 * NFB, 128)
          .transpose(0, 4, 1, 3, 2, 5).astype(np.float16))
    # w2r[e][j, p, :] = w2[e, j*128 + p, :]
    w2r = np.ascontiguousarray(w2.reshape(E, NFB, 128, H).astype(np.float16))
    # w28[e][pi, p, pl, :] = clip(S8 * w2[e, (j0+pl)*128 + p, :]) in e4m3
    w28 = np.zeros((E, max(N2, 1), 128, 2, H), dtype=NP8)
    for pi, (j0, j1) in enumerate(FP8_PAIRS):
        for pl, j in enumerate((j0, j1)):
            w28[:, pi, :, pl, :] = np.clip(
                S8 * w2.reshape(E, NFB, 128, H)[:, j], -240, 240).astype(NP8)
    return xt, w1t, w2r, w28


def run_cores(inputs, trace=False, **spmd_kwargs):
    xt, w1t, w2r, w28 = prep_inputs(
        inputs["permuted_hidden_states"], inputs["w1"], inputs["w2"])
    nc = _get_nc()
    in_maps = [{"xt": xt[e], "w1t": w1t[e], "w2r": w2r[e], "w28": w28[e]}
               for e in range(E)]
    res = run_bass_kernel_spmd(nc, in_maps, list(range(E)), trace=trace, **spmd_kwargs)
    outs = [
        res.results[e]["out"].reshape(NHB, 128, T).transpose(2, 0, 1).reshape(T, H)
        for e in range(E)
    ]
    full = np.concatenate(outs, axis=0).astype(np.float32)
    return full, res


def kernel(permuted_hidden_states, tokens_per_expert, w1, w2):
    full, _ = run_cores({
        "permuted_hidden_states": permuted_hidden_states,
        "w1": w1,
        "w2": w2,
    })
    return full
